# revision 2
# baseline (speedup 1.0000x reference)
"""CASCADES adapter (moe_routing) Trainium2 kernel.

Reference math:
    centroid = 0.7*x[:,-1,:] + 0.3*mean_s(x)           [B, IN]
    w        = softmax(cos(centroid, core_keys)/TEMP)  [B, K]
    Lam[b]   = sum_k w[b,k] * core_pool[k]             [B, R, R]
    out      = gate * x @ V^T @ Lam^T @ U^T            [B, S, OUT]
gate is a scalar depending only on U, V, gate_w, gate_b (host-computed).

Restructuring:
    out[b] = xV[b] @ UL[b]^T,   xV = x @ V^T (rank R=8),
    UL[b]  = gate * U @ Lam[b]  [OUT, R]  (tiny, host-computed)
Routing needs only per-batch column sums of x (device-computed in stage 1),
x[:,-1,:] and tiny tensors (host).

Sharding: 8 cores, core c owns batch c//2, S rows [(c%2)*2048, (c%2+1)*2048).
Stage 1 reads each x shard once (32 MB/core), stage 2 writes each output
shard once (32 MB/core) -> memory roofline ~64MB/core / ~360GB/s.

Precision: matmuls run as bf16 hi/lo "x3" decompositions (a@b ~= ah@bh +
ah@bl + al@bh, hi/lo split on host) -> ~1e-5 relative error at full bf16 PE
rate. Column sums accumulate in fp32 via ScalarE activation(accum_out=).
"""

import os
from contextlib import ExitStack

import ml_dtypes
import numpy as np

import concourse.bass as bass
import concourse.tile as tile
from concourse import bacc, mybir
from concourse.bass_utils import run_bass_kernel_spmd

FP = mybir.dt.float32
BF = mybir.dt.bfloat16
BF_NP = ml_dtypes.bfloat16

B, S, IN, OUT, R, K = 4, 4096, 4096, 4096, 8, 4
NCORES = 8
SSH = S // 2          # 2048: per-core S shard
NI_CH = IN // 128     # 32 contraction chunks
EPS = 1e-8
TEMP = 0.05

# Populated on every kernel() call when KERNEL_TRACE=1.
LAST_STATS: dict = {}

_prog_cache: dict = {}


def _split_hi_lo(a):
    """fp32 array -> (hi, lo) bf16 arrays with hi+lo ~= a (~16-bit mantissa)."""
    a = np.asarray(a, dtype=np.float32)
    hi = a.astype(BF_NP)
    lo = (a - hi.astype(np.float32)).astype(BF_NP)
    return hi, lo


def build_stage1():
    """Per core:
      xv[r, s] = sum_i V[r,i] * xT[i, s]    (bf16x3, fp32 PSUM accumulate)
      cs[p, ic] = sum_s (xh+xl)[ic*128+p, s]  (fp32 accumulate on ScalarE)
    Input xhl [IN, 2*SSH] bf16: row i = [xh_i (SSH) | xl_i (SSH)].
    Input vhl [128, 2*NI_CH*R] bf16: [Vh chunks (NI_CH*R) | Vl chunks].
    """
    nc = bacc.Bacc("TRN2", target_bir_lowering=False, debug=False, num_devices=NCORES)
    xhl = nc.dram_tensor("xhl", [IN, 2 * SSH], BF, kind="ExternalInput").ap()
    vhl = nc.dram_tensor("vhl", [128, 2 * NI_CH * R], BF, kind="ExternalInput").ap()
    xv = nc.dram_tensor("xv", [R, SSH], FP, kind="ExternalOutput").ap()
    cs = nc.dram_tensor("cs", [128, NI_CH], FP, kind="ExternalOutput").ap()

    with tile.TileContext(nc) as tc:
        with ExitStack() as ctx:
            xin = ctx.enter_context(tc.tile_pool(name="xin", bufs=6))
            scr = ctx.enter_context(tc.tile_pool(name="scr", bufs=2))
            small = ctx.enter_context(tc.tile_pool(name="small", bufs=1))
            psum = ctx.enter_context(tc.tile_pool(name="psum", bufs=1, space="PSUM"))

            v_sb = small.tile([128, 2 * NI_CH * R], BF)
            nc.sync.dma_start(v_sb[:], vhl[:])
            acc = small.tile([128, NI_CH], FP)
            xvp = psum.tile([R, SSH], FP)  # 4 PSUM banks, accumulated over all ic

            NSB = SSH // 512  # 4 rhs slices per pass
            for ic in range(NI_CH):
                xt = xin.tile([128, 2 * SSH], BF)
                nc.sync.dma_start(xt[:], xhl[ic * 128:(ic + 1) * 128, :])
                # column sums of (hi + lo), exact fp32 accumulation
                sc_t = scr.tile([128, 2 * SSH], BF)
                nc.scalar.activation(
                    sc_t[:], xt[:], mybir.ActivationFunctionType.Copy,
                    accum_out=acc[:, ic:ic + 1])
                # bf16x3: xh@Vh + xh@Vl + xl@Vh
                vh = v_sb[:, ic * R:(ic + 1) * R]
                vl = v_sb[:, NI_CH * R + ic * R: NI_CH * R + (ic + 1) * R]
                passes = [(vh, 0), (vl, 0), (vh, SSH)]
                for pi, (lhsT, roff) in enumerate(passes):
                    for sb in range(NSB):
                        nc.tensor.matmul(
                            xvp[:, sb * 512:(sb + 1) * 512],
                            lhsT,
                            xt[:, roff + sb * 512: roff + (sb + 1) * 512],
                            start=(ic == 0 and pi == 0),
                            stop=(ic == NI_CH - 1 and pi == len(passes) - 1),
                        )

            xv_sb = small.tile([R, SSH], FP)
            nc.vector.tensor_copy(xv_sb[:], xvp[:])
            nc.sync.dma_start(xv[:], xv_sb[:])
            nc.sync.dma_start(cs[:], acc[:])

    nc.compile()
    return nc


def build_stage2():
    """Per core: out[s, o] = sum_r xv[r, s] * ulT[r, o]  (bf16x3).
    Inputs xvhl [R, 2*SSH] bf16 ([xvh | xvl]), ulhl [R, 2*OUT] bf16."""
    nc = bacc.Bacc("TRN2", target_bir_lowering=False, debug=False, num_devices=NCORES)
    xvhl = nc.dram_tensor("xvhl", [R, 2 * SSH], BF, kind="ExternalInput").ap()
    ulhl = nc.dram_tensor("ulhl", [R, 2 * OUT], BF, kind="ExternalInput").ap()
    out = nc.dram_tensor("out", [SSH, OUT], FP, kind="ExternalOutput").ap()

    with tile.TileContext(nc) as tc:
        with ExitStack() as ctx:
            small = ctx.enter_context(tc.tile_pool(name="small", bufs=1))
            ostage = ctx.enter_context(tc.tile_pool(name="ostage", bufs=4))
            psum = ctx.enter_context(tc.tile_pool(name="psum", bufs=2, space="PSUM"))

            xv_sb = small.tile([R, 2 * SSH], BF)
            nc.sync.dma_start(xv_sb[:], xvhl[:])
            ul_sb = small.tile([R, 2 * OUT], BF)
            nc.sync.dma_start(ul_sb[:], ulhl[:])

            for sc in range(SSH // 128):       # 16 s-chunks
                xh = xv_sb[:, sc * 128:(sc + 1) * 128]
                xl = xv_sb[:, SSH + sc * 128: SSH + (sc + 1) * 128]
                for oh in range(OUT // 2048):  # 2 halves -> [128, 2048] tiles
                    op = psum.tile([128, 2048], FP)  # 4 banks
                    for ob in range(4):
                        o0 = oh * 2048 + ob * 512
                        uh = ul_sb[:, o0:o0 + 512]
                        ul = ul_sb[:, OUT + o0: OUT + o0 + 512]
                        for pi, (lhsT, rhs) in enumerate(
                                [(xh, uh), (xh, ul), (xl, uh)]):
                            nc.tensor.matmul(
                                op[:, ob * 512:(ob + 1) * 512], lhsT, rhs,
                                start=(pi == 0), stop=(pi == 2))
                    ot = ostage.tile([128, 2048], FP)
                    if (sc * 2 + oh) % 2 == 0:
                        nc.vector.tensor_copy(ot[:], op[:])
                    else:
                        nc.scalar.copy(ot[:], op[:])
                    nc.sync.dma_start(
                        out[sc * 128:(sc + 1) * 128, oh * 2048:(oh + 1) * 2048], ot[:])

    nc.compile()
    return nc


def _get_prog(name, builder):
    if name not in _prog_cache:
        _prog_cache[name] = builder()
    return _prog_cache[name]


def _routing_host(colsum, x_last, V_shared, U_shared, core_pool, core_keys,
                  gate_w, gate_b):
    """All tiny routing math in float64. colsum: [B, IN] sums over S.
    Returns UL[b] = gate * U @ Lam[b]  [B, OUT, R]."""
    m = colsum / S
    xl = x_last.astype(np.float64)
    centroid = 0.7 * xl + 0.3 * m
    cn = centroid / np.maximum(
        np.linalg.norm(centroid, axis=-1, keepdims=True), EPS)
    kn = core_keys.astype(np.float64)
    kn = kn / np.maximum(np.linalg.norm(kn, axis=-1, keepdims=True), EPS)
    sim = cn @ kn.T
    z = sim / TEMP
    z = z - z.max(axis=-1, keepdims=True)
    w = np.exp(z)
    w = w / w.sum(axis=-1, keepdims=True)
    Lam = np.einsum("bk,kij->bij", w, core_pool.astype(np.float64))
    gate_in = np.concatenate([
        U_shared.astype(np.float64).mean(axis=0),
        V_shared.astype(np.float64).mean(axis=1)])
    gate = 1.0 / (1.0 + np.exp(
        -(gate_w.astype(np.float64) @ gate_in + gate_b.astype(np.float64))))
    UL = gate[0] * np.einsum("oj,bjr->bor", U_shared.astype(np.float64), Lam)
    return UL


def kernel(x, V_shared, U_shared, core_pool, core_keys, gate_w, gate_b):
    trace = os.environ.get("KERNEL_TRACE", "") == "1"
    core_ids = list(range(NCORES))

    # ---- host prep: per-core transposed shards, split into bf16 hi/lo
    xhls = []
    for c in range(NCORES):
        xs = np.ascontiguousarray(x[c // 2, (c % 2) * SSH:(c % 2 + 1) * SSH, :].T)
        xh, xl = _split_hi_lo(xs)
        xhls.append(np.concatenate([xh, xl], axis=1))  # [IN, 2*SSH] bf16

    def chunk_major(vmat):  # [R, IN] -> [128, NI_CH*R]
        return np.ascontiguousarray(
            vmat.T.reshape(NI_CH, 128, R).transpose(1, 0, 2).reshape(128, NI_CH * R))

    vh, vl = _split_hi_lo(V_shared)
    vhl = np.concatenate(
        [chunk_major(vh.astype(np.float32)).astype(BF_NP),
         chunk_major(vl.astype(np.float32)).astype(BF_NP)], axis=1)

    # ---- stage 1 on device
    nc1 = _get_prog("s1", build_stage1)
    r1 = run_bass_kernel_spmd(
        nc1, [{"xhl": xhls[c], "vhl": vhl} for c in core_ids], core_ids, trace=trace)
    xvs = [r1.results[c]["xv"] for c in core_ids]
    css = [r1.results[c]["cs"] for c in core_ids]

    # ---- routing on host (tiny)
    colsum = np.stack([
        (css[2 * b].astype(np.float64) + css[2 * b + 1].astype(np.float64))
        .T.reshape(IN)
        for b in range(B)
    ])
    UL = _routing_host(colsum, x[:, -1, :], V_shared, U_shared, core_pool,
                       core_keys, gate_w, gate_b)

    # ---- stage 2 inputs: bf16 hi/lo splits of xv and UL^T
    xvhls, ulhls = [], []
    for c in range(NCORES):
        h, l = _split_hi_lo(xvs[c])
        xvhls.append(np.concatenate([h, l], axis=1))        # [R, 2*SSH]
        h, l = _split_hi_lo(np.ascontiguousarray(UL[c // 2].T.astype(np.float32)))
        ulhls.append(np.concatenate([h, l], axis=1))        # [R, 2*OUT]

    nc2 = _get_prog("s2", build_stage2)
    r2 = run_bass_kernel_spmd(
        nc2, [{"xvhl": xvhls[c], "ulhl": ulhls[c]} for c in core_ids], core_ids,
        trace=trace)
    outs = [r2.results[c]["out"] for c in core_ids]

    if trace:
        LAST_STATS.clear()
        LAST_STATS["stage1_ns"] = r1.exec_time_ns
        LAST_STATS["stage2_ns"] = r2.exec_time_ns
        LAST_STATS["total_ns"] = (
            (r1.exec_time_ns or 0) + (r2.exec_time_ns or 0)
            if (r1.exec_time_ns or r2.exec_time_ns) else None)

    return np.stack(
        [np.concatenate([outs[2 * b], outs[2 * b + 1]], axis=0) for b in range(B)]
    )


# revision 6
# speedup vs baseline: 1.1942x; 1.1942x over previous
"""CASCADES adapter (moe_routing) Trainium2 kernel.

Reference math:
    centroid = 0.7*x[:,-1,:] + 0.3*mean_s(x)           [B, IN]
    w        = softmax(cos(centroid, core_keys)/TEMP)  [B, K]
    Lam[b]   = sum_k w[b,k] * core_pool[k]             [B, R, R]
    out      = gate * x @ V^T @ Lam^T @ U^T            [B, S, OUT]
gate is a scalar depending only on U, V, gate_w, gate_b (host-computed).

Restructuring:
    out[b] = xV[b] @ UL[b]^T,   xV = x @ V^T (rank R=8),
    UL[b]  = gate * U @ Lam[b]  [OUT, R]  (tiny, host-computed)
Routing needs only per-batch column sums of x (device-computed in stage 1),
x[:,-1,:] and tiny tensors (host).

Sharding: 8 cores, core c owns batch c//2, S rows [(c%2)*2048, (c%2+1)*2048).
Stage 1 reads each x shard once (32 MB/core), stage 2 writes each output
shard once (32 MB/core) -> memory roofline ~64MB/core / ~360GB/s.

Precision: matmuls run as bf16 hi/lo "x3" decompositions (a@b ~= ah@bh +
ah@bl + al@bh, hi/lo split on host) -> ~1e-5 relative error at full bf16 PE
rate. Column sums accumulate in fp32 via ScalarE activation(accum_out=).
"""

import os
from contextlib import ExitStack

import ml_dtypes
import numpy as np

import concourse.bass as bass
import concourse.tile as tile
from concourse import bacc, mybir
from concourse.bass_utils import run_bass_kernel_spmd

FP = mybir.dt.float32
BF = mybir.dt.bfloat16
BF_NP = ml_dtypes.bfloat16

B, S, IN, OUT, R, K = 4, 4096, 4096, 4096, 8, 4
NCORES = 8
SSH = S // 2          # 2048: per-core S shard
NI_CH = IN // 128     # 32 contraction chunks
EPS = 1e-8
TEMP = 0.05

# Populated on every kernel() call when KERNEL_TRACE=1.
LAST_STATS: dict = {}

_prog_cache: dict = {}


def _split_hi_lo(a):
    """fp32 array -> (hi, lo) bf16 arrays with hi+lo ~= a (~16-bit mantissa)."""
    a = np.asarray(a, dtype=np.float32)
    hi = a.astype(BF_NP)
    lo = (a - hi.astype(np.float32)).astype(BF_NP)
    return hi, lo


def build_stage1():
    """Per core:
      xv[r, s] = sum_i V[r,i] * xT[i, s]    (bf16x3, fp32 PSUM accumulate)
      cs[p, ic] = sum_s (xh+xl)[ic*128+p, s]  (fp32 accumulate on ScalarE)
    Input xhl [IN, 2*SSH] bf16: row i = [xh_i (SSH) | xl_i (SSH)].
    Input vhl [128, 2*NI_CH*R] bf16: [Vh chunks (NI_CH*R) | Vl chunks].
    """
    nc = bacc.Bacc("TRN2", target_bir_lowering=False, debug=False, num_devices=NCORES)
    xhl = nc.dram_tensor("xhl", [IN, 2 * SSH], BF, kind="ExternalInput").ap()
    vhl = nc.dram_tensor("vhl", [128, 2 * NI_CH * R], BF, kind="ExternalInput").ap()
    xv = nc.dram_tensor("xv", [R, SSH], FP, kind="ExternalOutput").ap()
    cs = nc.dram_tensor("cs", [128, 2 * NI_CH], FP, kind="ExternalOutput").ap()

    with tile.TileContext(nc) as tc:
        with ExitStack() as ctx:
            xin = ctx.enter_context(tc.tile_pool(name="xin", bufs=6))
            scr = ctx.enter_context(tc.tile_pool(name="scr", bufs=2))
            scr2 = ctx.enter_context(tc.tile_pool(name="scr2", bufs=2))
            small = ctx.enter_context(tc.tile_pool(name="small", bufs=1))
            psum = ctx.enter_context(tc.tile_pool(name="psum", bufs=1, space="PSUM"))

            v_sb = small.tile([128, 2 * NI_CH * R], BF)
            nc.sync.dma_start(v_sb[:], vhl[:])
            acc = small.tile([128, 2 * NI_CH], FP)  # [hi sums | lo sums]
            xvp = psum.tile([R, SSH], FP)  # 4 PSUM banks, accumulated over all ic

            NSB = SSH // 512  # 4 rhs slices per pass
            for ic in range(NI_CH):
                xt = xin.tile([128, 2 * SSH], BF)
                nc.sync.dma_start(xt[:], xhl[ic * 128:(ic + 1) * 128, :])
                # column sums: hi half on ScalarE, lo half on VectorE (both
                # 1x-mode ops, so split across engines); host adds them.
                sc_t = scr.tile([128, SSH], BF)
                nc.scalar.activation(
                    sc_t[:], xt[:, :SSH], mybir.ActivationFunctionType.Copy,
                    accum_out=acc[:, ic:ic + 1])
                sc_t2 = scr2.tile([128, SSH], BF)
                nc.vector.tensor_scalar(
                    sc_t2[:], xt[:, SSH:], 1.0, None, mybir.AluOpType.mult,
                    mybir.AluOpType.add,
                    accum_out=acc[:, NI_CH + ic:NI_CH + ic + 1])
                # bf16x3: xh@Vh + xh@Vl + xl@Vh
                vh = v_sb[:, ic * R:(ic + 1) * R]
                vl = v_sb[:, NI_CH * R + ic * R: NI_CH * R + (ic + 1) * R]
                passes = [(vh, 0), (vl, 0), (vh, SSH)]
                for pi, (lhsT, roff) in enumerate(passes):
                    for sb in range(NSB):
                        nc.tensor.matmul(
                            xvp[:, sb * 512:(sb + 1) * 512],
                            lhsT,
                            xt[:, roff + sb * 512: roff + (sb + 1) * 512],
                            start=(ic == 0 and pi == 0),
                            stop=(ic == NI_CH - 1 and pi == len(passes) - 1),
                        )

            xv_sb = small.tile([R, SSH], FP)
            nc.vector.tensor_copy(xv_sb[:], xvp[:])
            nc.sync.dma_start(xv[:], xv_sb[:])
            nc.sync.dma_start(cs[:], acc[:])

    nc.compile()
    return nc


def build_stage2():
    """Per core: out[s, o] = sum_r xv[r, s] * ulT[r, o]  (bf16x3).

    Inputs are replicated into all four 32-partition quadrants so matmuls
    can rotate PE row groups (tile_position) - 4 concurrent 32x128 tiles
    hide the per-matmul LDWEIGHTS that otherwise serializes (K=8).
      xvq [128, 2*SSH] bf16: rows 32q..32q+7 = [xvh | xvl]
      ulq [128, 2*OUT] bf16: rows 32q..32q+7 = [ulh | ull]
    """
    nc = bacc.Bacc("TRN2", target_bir_lowering=False, debug=False, num_devices=NCORES)
    xvq = nc.dram_tensor("xvq", [128, 2 * SSH], BF, kind="ExternalInput").ap()
    ulq = nc.dram_tensor("ulq", [128, 2 * OUT], BF, kind="ExternalInput").ap()
    out = nc.dram_tensor("out", [SSH, OUT], FP, kind="ExternalOutput").ap()

    with tile.TileContext(nc) as tc:
        with ExitStack() as ctx:
            small = ctx.enter_context(tc.tile_pool(name="small", bufs=1))
            ostage = ctx.enter_context(tc.tile_pool(name="ostage", bufs=4))
            psum = ctx.enter_context(tc.tile_pool(name="psum", bufs=2, space="PSUM"))

            xv_sb = small.tile([128, 2 * SSH], BF)
            nc.sync.dma_start(xv_sb[:], xvq[:])
            ul_sb = small.tile([128, 2 * OUT], BF)
            nc.sync.dma_start(ul_sb[:], ulq[:])

            for sc in range(SSH // 128):       # 16 s-chunks
                for oh in range(OUT // 2048):  # 2 halves -> [128, 2048] tiles
                    q = (sc * 2 + oh) % 4      # PE row group for this tile
                    p0 = 32 * q
                    xh = xv_sb[p0:p0 + R, sc * 128:(sc + 1) * 128]
                    xl = xv_sb[p0:p0 + R, SSH + sc * 128: SSH + (sc + 1) * 128]
                    op = psum.tile([128, 2048], FP)  # 4 banks
                    for ob in range(4):
                        o0 = oh * 2048 + ob * 512
                        uh = ul_sb[p0:p0 + R, o0:o0 + 512]
                        ul = ul_sb[p0:p0 + R, OUT + o0: OUT + o0 + 512]
                        for pi, (lhsT, rhs) in enumerate(
                                [(xh, uh), (xh, ul), (xl, uh)]):
                            nc.tensor.matmul(
                                op[:, ob * 512:(ob + 1) * 512], lhsT, rhs,
                                start=(pi == 0), stop=(pi == 2),
                                tile_position=(p0, 0))
                    ot = ostage.tile([128, 2048], FP)
                    if (sc * 2 + oh) % 2 == 0:
                        nc.vector.tensor_copy(ot[:], op[:])
                    else:
                        nc.scalar.copy(ot[:], op[:])
                    nc.sync.dma_start(
                        out[sc * 128:(sc + 1) * 128, oh * 2048:(oh + 1) * 2048], ot[:])

    nc.compile()
    return nc


def _get_prog(name, builder):
    if name not in _prog_cache:
        _prog_cache[name] = builder()
    return _prog_cache[name]


def _routing_host(colsum, x_last, V_shared, U_shared, core_pool, core_keys,
                  gate_w, gate_b):
    """All tiny routing math in float64. colsum: [B, IN] sums over S.
    Returns UL[b] = gate * U @ Lam[b]  [B, OUT, R]."""
    m = colsum / S
    xl = x_last.astype(np.float64)
    centroid = 0.7 * xl + 0.3 * m
    cn = centroid / np.maximum(
        np.linalg.norm(centroid, axis=-1, keepdims=True), EPS)
    kn = core_keys.astype(np.float64)
    kn = kn / np.maximum(np.linalg.norm(kn, axis=-1, keepdims=True), EPS)
    sim = cn @ kn.T
    z = sim / TEMP
    z = z - z.max(axis=-1, keepdims=True)
    w = np.exp(z)
    w = w / w.sum(axis=-1, keepdims=True)
    Lam = np.einsum("bk,kij->bij", w, core_pool.astype(np.float64))
    gate_in = np.concatenate([
        U_shared.astype(np.float64).mean(axis=0),
        V_shared.astype(np.float64).mean(axis=1)])
    gate = 1.0 / (1.0 + np.exp(
        -(gate_w.astype(np.float64) @ gate_in + gate_b.astype(np.float64))))
    UL = gate[0] * np.einsum("oj,bjr->bor", U_shared.astype(np.float64), Lam)
    return UL


def kernel(x, V_shared, U_shared, core_pool, core_keys, gate_w, gate_b):
    trace = os.environ.get("KERNEL_TRACE", "") == "1"
    core_ids = list(range(NCORES))

    # ---- host prep: per-core transposed shards, split into bf16 hi/lo
    xhls = []
    for c in range(NCORES):
        xs = np.ascontiguousarray(x[c // 2, (c % 2) * SSH:(c % 2 + 1) * SSH, :].T)
        xh, xl = _split_hi_lo(xs)
        xhls.append(np.concatenate([xh, xl], axis=1))  # [IN, 2*SSH] bf16

    def chunk_major(vmat):  # [R, IN] -> [128, NI_CH*R]
        return np.ascontiguousarray(
            vmat.T.reshape(NI_CH, 128, R).transpose(1, 0, 2).reshape(128, NI_CH * R))

    vh, vl = _split_hi_lo(V_shared)
    vhl = np.concatenate(
        [chunk_major(vh.astype(np.float32)).astype(BF_NP),
         chunk_major(vl.astype(np.float32)).astype(BF_NP)], axis=1)

    # ---- stage 1 on device
    nc1 = _get_prog("s1", build_stage1)
    r1 = run_bass_kernel_spmd(
        nc1, [{"xhl": xhls[c], "vhl": vhl} for c in core_ids], core_ids, trace=trace)
    xvs = [r1.results[c]["xv"] for c in core_ids]
    css = [r1.results[c]["cs"] for c in core_ids]

    # ---- routing on host (tiny); cs = [hi sums | lo sums], add both halves
    def core_colsum(csm):
        m = csm.astype(np.float64)
        return (m[:, :NI_CH] + m[:, NI_CH:]).T.reshape(IN)

    colsum = np.stack([
        core_colsum(css[2 * b]) + core_colsum(css[2 * b + 1]) for b in range(B)
    ])
    UL = _routing_host(colsum, x[:, -1, :], V_shared, U_shared, core_pool,
                       core_keys, gate_w, gate_b)

    # ---- stage 2 inputs: bf16 hi/lo splits, replicated into the 4 partition
    # quadrants for PE row-group rotation
    def quad(hl):  # [R, W] -> [128, W] with rows 32q..32q+7 = hl
        outq = np.zeros((128, hl.shape[1]), dtype=BF_NP)
        for q in range(4):
            outq[32 * q:32 * q + R] = hl
        return outq

    xvqs, ulqs = [], []
    for c in range(NCORES):
        h, l = _split_hi_lo(xvs[c])
        xvqs.append(quad(np.concatenate([h, l], axis=1)))    # [128, 2*SSH]
        h, l = _split_hi_lo(np.ascontiguousarray(UL[c // 2].T.astype(np.float32)))
        ulqs.append(quad(np.concatenate([h, l], axis=1)))    # [128, 2*OUT]

    nc2 = _get_prog("s2", build_stage2)
    r2 = run_bass_kernel_spmd(
        nc2, [{"xvq": xvqs[c], "ulq": ulqs[c]} for c in core_ids], core_ids,
        trace=trace)
    outs = [r2.results[c]["out"] for c in core_ids]

    if trace:
        LAST_STATS.clear()
        LAST_STATS["stage1_ns"] = r1.exec_time_ns
        LAST_STATS["stage2_ns"] = r2.exec_time_ns
        LAST_STATS["total_ns"] = (
            (r1.exec_time_ns or 0) + (r2.exec_time_ns or 0)
            if (r1.exec_time_ns or r2.exec_time_ns) else None)

    return np.stack(
        [np.concatenate([outs[2 * b], outs[2 * b + 1]], axis=0) for b in range(B)]
    )


# revision 11
# speedup vs baseline: 1.4239x; 1.1923x over previous
"""CASCADES adapter (moe_routing) Trainium2 kernel.

Reference math:
    centroid = 0.7*x[:,-1,:] + 0.3*mean_s(x)           [B, IN]
    w        = softmax(cos(centroid, core_keys)/TEMP)  [B, K]
    Lam[b]   = sum_k w[b,k] * core_pool[k]             [B, R, R]
    out      = gate * x @ V^T @ Lam^T @ U^T            [B, S, OUT]
gate is a scalar depending only on U, V, gate_w, gate_b (host-computed).

Restructuring:
    out[b] = xV[b] @ UL[b]^T,   xV = x @ V^T (rank R=8),
    UL[b]  = gate * U @ Lam[b]  [OUT, R]  (tiny, host-computed)
Routing needs only per-batch column sums of x (device-computed in stage 1),
x[:,-1,:] and tiny tensors (host).

Sharding: 8 cores, core c owns batch c//2, S rows [(c%2)*2048, (c%2+1)*2048).
Stage 1 reads each x shard once (32 MB/core), stage 2 writes each output
shard once (32 MB/core) -> memory roofline ~64MB/core / ~360GB/s.

Precision: matmuls run as bf16 hi/lo "x3" decompositions (a@b ~= ah@bh +
ah@bl + al@bh, hi/lo split on host) -> ~1e-5 relative error at full bf16 PE
rate. Column sums accumulate in fp32 via ScalarE activation(accum_out=).
"""

import os
from contextlib import ExitStack

import ml_dtypes
import numpy as np

import concourse.bass as bass
import concourse.tile as tile
from concourse import bacc, mybir
from concourse.bass_utils import run_bass_kernel_spmd

FP = mybir.dt.float32
BF = mybir.dt.bfloat16
BF_NP = ml_dtypes.bfloat16

B, S, IN, OUT, R, K = 4, 4096, 4096, 4096, 8, 4
NCORES = 8
SSH = S // 2          # 2048: per-core S shard
NI_CH = IN // 128     # 32 contraction chunks
EPS = 1e-8
TEMP = 0.05

# Populated on every kernel() call when KERNEL_TRACE=1.
LAST_STATS: dict = {}

_prog_cache: dict = {}


def _split_hi_lo(a):
    """fp32 array -> (hi, lo) bf16 arrays with hi+lo ~= a (~16-bit mantissa)."""
    a = np.asarray(a, dtype=np.float32)
    hi = a.astype(BF_NP)
    lo = (a - hi.astype(np.float32)).astype(BF_NP)
    return hi, lo


def build_stage1():
    """Per core:
      xv[r, s] = sum_i V[r,i] * xT[i, s]    (bf16x3, fp32 PSUM accumulate)
      cs[p, ic] = sum_s (xh+xl)[ic*128+p, s]  (fp32 accumulate on ScalarE)
    Input xhl [IN, 2*SSH] bf16: row i = [xh_i (SSH) | xl_i (SSH)].
    Input vhl [128, 2*NI_CH*R] bf16: [Vh chunks (NI_CH*R) | Vl chunks].
    """
    nc = bacc.Bacc("TRN2", target_bir_lowering=False, debug=False, num_devices=NCORES)
    xhl = nc.dram_tensor("xhl", [IN, 2 * SSH], BF, kind="ExternalInput").ap()
    vhl = nc.dram_tensor("vhl", [128, 2 * NI_CH * R], BF, kind="ExternalInput").ap()
    xv = nc.dram_tensor("xv", [R, 4 * 512], FP, kind="ExternalOutput").ap()
    cs = nc.dram_tensor("cs", [128, 2 * NI_CH], FP, kind="ExternalOutput").ap()

    with tile.TileContext(nc) as tc:
        with ExitStack() as ctx:
            xin = ctx.enter_context(tc.tile_pool(name="xin", bufs=8))
            scr = ctx.enter_context(tc.tile_pool(name="scr", bufs=2))
            scr2 = ctx.enter_context(tc.tile_pool(name="scr2", bufs=2))
            small = ctx.enter_context(tc.tile_pool(name="small", bufs=1))
            psum = ctx.enter_context(tc.tile_pool(name="psum", bufs=1, space="PSUM"))

            v_sb = small.tile([128, 2 * NI_CH * R], BF)
            nc.sync.dma_start(v_sb[:], vhl[:])
            acc = small.tile([128, 2 * NI_CH], FP)  # [hi sums | lo sums]
            # s-slice sb accumulates at partitions 32*sb..+8, bank sb
            # (PE column tiling: 4 concurrent 128x32 tiles; one accumulation
            # group per PSUM bank - groups are bank-granular).
            xvp = psum.tile([128, 4 * 512], FP)

            NSB = SSH // 512  # 4 rhs slices per pass
            for ic in range(NI_CH):
                xt = xin.tile([128, 2 * SSH], BF)
                nc.sync.dma_start(xt[:], xhl[ic * 128:(ic + 1) * 128, :])
                # column sums: hi half on ScalarE, lo half on VectorE (both
                # 1x-mode ops, so split across engines); host adds them.
                sc_t = scr.tile([128, SSH], BF)
                nc.scalar.activation(
                    sc_t[:], xt[:, :SSH], mybir.ActivationFunctionType.Copy,
                    accum_out=acc[:, ic:ic + 1])
                sc_t2 = scr2.tile([128, SSH], BF)
                nc.vector.tensor_scalar(
                    sc_t2[:], xt[:, SSH:], 1.0, None, mybir.AluOpType.mult,
                    mybir.AluOpType.add,
                    accum_out=acc[:, NI_CH + ic:NI_CH + ic + 1])
                # bf16x3: xh@Vh + xh@Vl + xl@Vh; sb rotates PE column group
                vh = v_sb[:, ic * R:(ic + 1) * R]
                vl = v_sb[:, NI_CH * R + ic * R: NI_CH * R + (ic + 1) * R]
                passes = [(vh, 0), (vl, 0), (vh, SSH)]
                for pi, (lhsT, roff) in enumerate(passes):
                    for sb in range(NSB):
                        nc.tensor.matmul(
                            xvp[32 * sb:32 * sb + R, sb * 512:(sb + 1) * 512],
                            lhsT,
                            xt[:, roff + sb * 512: roff + (sb + 1) * 512],
                            start=(ic == 0 and pi == 0),
                            stop=(ic == NI_CH - 1 and pi == len(passes) - 1),
                            tile_position=(0, 32 * sb),
                        )

            xv_sb = small.tile([R, 4 * 512], FP)
            for sb in range(NSB):
                nc.vector.tensor_copy(
                    xv_sb[:, sb * 512:(sb + 1) * 512],
                    xvp[32 * sb:32 * sb + R, sb * 512:(sb + 1) * 512])
            nc.sync.dma_start(xv[:], xv_sb[:])
            nc.sync.dma_start(cs[:], acc[:])

    nc.compile()
    return nc


def build_stage2():
    """Per core: out[s, o] = sum_r xv[r, s] * ulT[r, o]  (bf16x3).

    Inputs are replicated into all four 32-partition quadrants so matmuls
    can rotate PE row groups (tile_position) - 4 concurrent 32x128 tiles
    hide the per-matmul LDWEIGHTS that otherwise serializes (K=8).
      xvq [128, 2*SSH] bf16: rows 32q..32q+7 = [xvh | xvl]
      ulq [128, 2*OUT] bf16: rows 32q..32q+7 = [ulh | ull]
    """
    nc = bacc.Bacc("TRN2", target_bir_lowering=False, debug=False, num_devices=NCORES)
    xvq = nc.dram_tensor("xvq", [128, 2 * SSH], BF, kind="ExternalInput").ap()
    ulq = nc.dram_tensor("ulq", [128, 2 * OUT], BF, kind="ExternalInput").ap()
    out = nc.dram_tensor("out", [SSH, OUT], FP, kind="ExternalOutput").ap()

    with tile.TileContext(nc) as tc:
        with ExitStack() as ctx:
            small = ctx.enter_context(tc.tile_pool(name="small", bufs=1))
            ostage = ctx.enter_context(tc.tile_pool(name="ostage", bufs=4))
            psum = ctx.enter_context(tc.tile_pool(name="psum", bufs=2, space="PSUM"))

            xv_sb = small.tile([128, 2 * SSH], BF)
            nc.sync.dma_start(xv_sb[:], xvq[:])
            ul_sb = small.tile([128, 2 * OUT], BF)
            nc.sync.dma_start(ul_sb[:], ulq[:])

            for sc in range(SSH // 128):       # 16 s-chunks
                for oh in range(OUT // 2048):  # 2 halves -> [128, 2048] tiles
                    op = psum.tile([128, 2048], FP)  # 4 banks
                    # pass-major, ob rotates the PE row group every matmul so
                    # LDWEIGHTS+streams of adjacent matmuls overlap
                    for pi in range(3):
                        for ob in range(4):
                            p0 = 32 * ob
                            xh = xv_sb[p0:p0 + R, sc * 128:(sc + 1) * 128]
                            xl = xv_sb[p0:p0 + R, SSH + sc * 128: SSH + (sc + 1) * 128]
                            o0 = oh * 2048 + ob * 512
                            uh = ul_sb[p0:p0 + R, o0:o0 + 512]
                            ul = ul_sb[p0:p0 + R, OUT + o0: OUT + o0 + 512]
                            lhsT, rhs = [(xh, uh), (xh, ul), (xl, uh)][pi]
                            nc.tensor.matmul(
                                op[:, ob * 512:(ob + 1) * 512], lhsT, rhs,
                                start=(pi == 0), stop=(pi == 2),
                                tile_position=(p0, 0))
                    ot = ostage.tile([128, 2048], FP)
                    if (sc * 2 + oh) % 2 == 0:
                        nc.vector.tensor_copy(ot[:], op[:])
                    else:
                        nc.scalar.copy(ot[:], op[:])
                    nc.sync.dma_start(
                        out[sc * 128:(sc + 1) * 128, oh * 2048:(oh + 1) * 2048], ot[:])

    nc.compile()
    return nc


def _get_prog(name, builder):
    if name not in _prog_cache:
        _prog_cache[name] = builder()
    return _prog_cache[name]


def _routing_host(colsum, x_last, V_shared, U_shared, core_pool, core_keys,
                  gate_w, gate_b):
    """All tiny routing math in float64. colsum: [B, IN] sums over S.
    Returns UL[b] = gate * U @ Lam[b]  [B, OUT, R]."""
    m = colsum / S
    xl = x_last.astype(np.float64)
    centroid = 0.7 * xl + 0.3 * m
    cn = centroid / np.maximum(
        np.linalg.norm(centroid, axis=-1, keepdims=True), EPS)
    kn = core_keys.astype(np.float64)
    kn = kn / np.maximum(np.linalg.norm(kn, axis=-1, keepdims=True), EPS)
    sim = cn @ kn.T
    z = sim / TEMP
    z = z - z.max(axis=-1, keepdims=True)
    w = np.exp(z)
    w = w / w.sum(axis=-1, keepdims=True)
    Lam = np.einsum("bk,kij->bij", w, core_pool.astype(np.float64))
    gate_in = np.concatenate([
        U_shared.astype(np.float64).mean(axis=0),
        V_shared.astype(np.float64).mean(axis=1)])
    gate = 1.0 / (1.0 + np.exp(
        -(gate_w.astype(np.float64) @ gate_in + gate_b.astype(np.float64))))
    UL = gate[0] * np.einsum("oj,bjr->bor", U_shared.astype(np.float64), Lam)
    return UL


def kernel(x, V_shared, U_shared, core_pool, core_keys, gate_w, gate_b):
    trace = os.environ.get("KERNEL_TRACE", "") == "1"
    core_ids = list(range(NCORES))

    # ---- host prep: per-core transposed shards, split into bf16 hi/lo
    xhls = []
    for c in range(NCORES):
        xs = np.ascontiguousarray(x[c // 2, (c % 2) * SSH:(c % 2 + 1) * SSH, :].T)
        xh, xl = _split_hi_lo(xs)
        xhls.append(np.concatenate([xh, xl], axis=1))  # [IN, 2*SSH] bf16

    def chunk_major(vmat):  # [R, IN] -> [128, NI_CH*R]
        return np.ascontiguousarray(
            vmat.T.reshape(NI_CH, 128, R).transpose(1, 0, 2).reshape(128, NI_CH * R))

    vh, vl = _split_hi_lo(V_shared)
    vhl = np.concatenate(
        [chunk_major(vh.astype(np.float32)).astype(BF_NP),
         chunk_major(vl.astype(np.float32)).astype(BF_NP)], axis=1)

    # ---- stage 1 on device
    nc1 = _get_prog("s1", build_stage1)
    r1 = run_bass_kernel_spmd(
        nc1, [{"xhl": xhls[c], "vhl": vhl} for c in core_ids], core_ids, trace=trace)
    xvs = [r1.results[c]["xv"] for c in core_ids]  # [R, SSH]
    css = [r1.results[c]["cs"] for c in core_ids]

    # ---- routing on host (tiny); cs = [hi sums | lo sums], add both halves
    def core_colsum(csm):
        m = csm.astype(np.float64)
        return (m[:, :NI_CH] + m[:, NI_CH:]).T.reshape(IN)

    colsum = np.stack([
        core_colsum(css[2 * b]) + core_colsum(css[2 * b + 1]) for b in range(B)
    ])
    UL = _routing_host(colsum, x[:, -1, :], V_shared, U_shared, core_pool,
                       core_keys, gate_w, gate_b)

    # ---- stage 2 inputs: bf16 hi/lo splits, replicated into the 4 partition
    # quadrants for PE row-group rotation
    def quad(hl):  # [R, W] -> [128, W] with rows 32q..32q+7 = hl
        outq = np.zeros((128, hl.shape[1]), dtype=BF_NP)
        for q in range(4):
            outq[32 * q:32 * q + R] = hl
        return outq

    xvqs, ulqs = [], []
    for c in range(NCORES):
        h, l = _split_hi_lo(xvs[c])
        xvqs.append(quad(np.concatenate([h, l], axis=1)))    # [128, 2*SSH]
        h, l = _split_hi_lo(np.ascontiguousarray(UL[c // 2].T.astype(np.float32)))
        ulqs.append(quad(np.concatenate([h, l], axis=1)))    # [128, 2*OUT]

    nc2 = _get_prog("s2", build_stage2)
    r2 = run_bass_kernel_spmd(
        nc2, [{"xvq": xvqs[c], "ulq": ulqs[c]} for c in core_ids], core_ids,
        trace=trace)
    outs = [r2.results[c]["out"] for c in core_ids]

    if trace:
        LAST_STATS.clear()
        LAST_STATS["stage1_ns"] = r1.exec_time_ns
        LAST_STATS["stage2_ns"] = r2.exec_time_ns
        LAST_STATS["total_ns"] = (
            (r1.exec_time_ns or 0) + (r2.exec_time_ns or 0)
            if (r1.exec_time_ns or r2.exec_time_ns) else None)

    return np.stack(
        [np.concatenate([outs[2 * b], outs[2 * b + 1]], axis=0) for b in range(B)]
    )
